# revision 2
# baseline (speedup 1.0000x reference)
"""Sparse attention (per-query top-K) Trainium2 kernel, 8-core tensor-parallel.

Strategy (heads sharded 2-per-core, dense-score formulation):
  - Host folds idx/valid/geo_bias into per-(s,q) merged bias factors
    E[s,q] = sum_{j: idx[q,j]==s} exp(geo_bias[h,q,j]), stored as causal
    fp16 tiles.  This turns the per-query gather/softmax into dense math:
        A^T = E^T * exp(S^T - C),   S^T = K @ Q^T (feature-major)
        out^T = [V | 1]^T @ A^T     (row 64 = softmax denominator)
  - Per core: Q/K/V projections for its 2 heads (x pre-cast to bf16 on
    host, transposed on load by the DMA xbar; projections in bf16 with
    fp32 accumulation), dense causal S^T on PE (fp32r), exp on ACT
    (both heads per instruction via a 2-bank PSUM tile), E-multiply on
    DVE (fp16), AV on PE (fp16).
  - AllToAll reshards the (unnormalized) head outputs + denominators so
    each core owns 512 query rows with all 16 heads, normalizes, and
    computes its o_proj row-slice with the full Wo.  Host concatenates.
"""

import sys

sys.path.insert(0, "/opt/trn_rl_repo")

import numpy as np
import ml_dtypes

from concourse import bacc, mybir, tile
from concourse.bass_utils import run_bass_kernel_spmd
from concourse.masks import make_identity

F32 = mybir.dt.float32
F32R = mybir.dt.float32r
F16 = mybir.dt.float16
BF16 = mybir.dt.bfloat16

S = 4096
H = 1024
NH = 16
KSEL = 32
HD = 64
NC = 8
HPC = NH // NC  # 2 heads per core
QT = 512
NQT = S // QT
SC = 128
CSHIFT = 2.0
SLAB = 16  # s-chunks per E-tile DMA slab

TILE_LIST = [(t, c) for t in range(NQT) for c in range(4 * (t + 1))]
N_TILES = len(TILE_LIST)  # 144
TILE_IDX = {tc: n for n, tc in enumerate(TILE_LIST)}


def _build_program(phases=3, n_reps=1, n_cores_build=NC):
    nc = bacc.Bacc(
        "TRN2", target_bir_lowering=False, debug=False, num_devices=n_cores_build
    )

    x_in = nc.dram_tensor("x", [S, H], BF16, kind="ExternalInput").ap()
    wq_in = nc.dram_tensor("wq", [H, 128], BF16, kind="ExternalInput").ap()
    wk_in = nc.dram_tensor("wk", [H, 128], BF16, kind="ExternalInput").ap()
    wv_in = nc.dram_tensor("wv", [H, 128], BF16, kind="ExternalInput").ap()
    wo_in = nc.dram_tensor("wo", [H, H], F32R, kind="ExternalInput").ap()
    bo_in = nc.dram_tensor("bo_rep", [128, H], F32, kind="ExternalInput").ap()
    e_in = nc.dram_tensor(
        "e_pack", [N_TILES, SC, HPC, QT], F16, kind="ExternalInput"
    ).ap()
    sel_in = nc.dram_tensor("sel16", [NH, H], F32, kind="ExternalInput").ap()
    y_out = nc.dram_tensor("y_part", [QT, H], F32, kind="ExternalOutput").ap()

    with tile.TileContext(nc) as tc:
        with (
            tc.tile_pool(name="const", bufs=1) as constp,
            tc.tile_pool(name="persist", bufs=1) as persist,
            tc.tile_pool(name="dram", bufs=1, space="DRAM") as dram,
        ):
            ident = constp.tile([128, 128], F32, tag="ident")
            make_identity(nc, ident[:])
            nbias = constp.tile([128, 1], F32, tag="nbias")
            nc.gpsimd.memset(nbias[:], -CSHIFT)

            wq_sb = constp.tile([128, 8, 128], BF16, tag="wq")
            wk_sb = constp.tile([128, 8, 128], BF16, tag="wk")
            wv_sb = constp.tile([128, 8, 128], BF16, tag="wv")
            nc.sync.dma_start(wq_sb[:], wq_in.rearrange("(c p) m -> p c m", p=128))
            nc.sync.dma_start(wk_sb[:], wk_in.rearrange("(c p) m -> p c m", p=128))
            nc.sync.dma_start(wv_sb[:], wv_in.rearrange("(c p) m -> p c m", p=128))
            wo_sb = constp.tile([128, 8, H], F32R, tag="wo")
            nc.sync.dma_start(wo_sb[:], wo_in.rearrange("(c p) f -> p c f", p=128))
            bo_sb = constp.tile([128, H], F32, tag="bo")
            nc.sync.dma_start(bo_sb[:], bo_in[:])
            sel_sb = constp.tile([NH, H], F32, tag="sel")
            nc.sync.dma_start(sel_sb[:], sel_in[:])

            qT_sb = persist.tile([128, NQT, QT], F32R, tag="qT")
            kT_sb = persist.tile([128, NQT, QT], F32R, tag="kT")
            v_sb = [
                persist.tile([128, S // SC, HD + 1], F16, tag=f"v{h}", name=f"v{h}")
                for h in range(HPC)
            ]
            for h in range(HPC):
                nc.gpsimd.memset(v_sb[h][:], 1.0)

            a2a_in = dram.tile([NC, HPC * (HD + 1), QT], F16)
            a2a_out = dram.tile([NC, HPC * (HD + 1), QT], F16)

            for _rep in range(n_reps):
                # ------------- phase 1: projections (feature-major) ---------
                with (
                    tc.tile_pool(name="xT", bufs=1) as xTp,
                    tc.tile_pool(name="vtmp", bufs=2) as vtmpp,
                    tc.tile_pool(name="p1ps", bufs=3, space="PSUM") as p1ps,
                    tc.tile_pool(name="p1projps", bufs=1, space="PSUM") as p1pp,
                ):
                    xT_full = xTp.tile([128, 8, S], BF16, tag="xTf")
                    for hc in range(8):
                        nc.sync.dma_start_transpose(
                            xT_full[:, hc, :], x_in[:, hc * 128 : (hc + 1) * 128]
                        )
                    for st in range(NQT):
                        sl = slice(st * QT, (st + 1) * QT)
                        ps_q = p1pp.tile([128, QT], F32, tag="psq")
                        ps_k = p1pp.tile([128, QT], F32, tag="psk")
                        ps_v = p1pp.tile([128, QT], F32, tag="psv")
                        for c in range(8):
                            nc.tensor.matmul(
                                ps_q[:], wq_sb[:, c, :], xT_full[:, c, sl],
                                start=(c == 0), stop=(c == 7),
                            )
                            nc.tensor.matmul(
                                ps_k[:], wk_sb[:, c, :], xT_full[:, c, sl],
                                start=(c == 0), stop=(c == 7),
                            )
                            nc.tensor.matmul(
                                ps_v[:], wv_sb[:, c, :], xT_full[:, c, sl],
                                start=(c == 0), stop=(c == 7),
                            )
                        nc.vector.tensor_copy(qT_sb[:, st, :], ps_q[:])
                        nc.vector.tensor_copy(kT_sb[:, st, :], ps_k[:])
                        vT_tmp = vtmpp.tile([128, QT], F32, tag="vt")
                        nc.scalar.copy(vT_tmp[:], ps_v[:])
                        ps_tv = p1ps.tile([128, QT], F32, tag="tp")
                        for i in range(4):
                            nc.tensor.transpose(
                                ps_tv[:, i * 128 : (i + 1) * 128],
                                vT_tmp[:, i * 128 : (i + 1) * 128],
                                ident[:],
                            )
                        ps_tv4 = ps_tv[:].rearrange("p (i h d) -> p i h d", i=4, h=HPC)
                        for h in range(HPC):
                            nc.vector.tensor_copy(
                                v_sb[h][:, st * 4 : (st + 1) * 4, 0:HD],
                                ps_tv4[:, :, h, :],
                            )

                # ------------- phase 2: attention ---------------------------
                if phases >= 2:
                    with (
                        tc.tile_pool(name="zap", bufs=6) as zap,
                        tc.tile_pool(name="ep", bufs=2) as epool,
                        tc.tile_pool(name="otp", bufs=2) as otp,
                        tc.tile_pool(name="p2s", bufs=3, space="PSUM") as p2s,
                        tc.tile_pool(name="p2o", bufs=1, space="PSUM") as p2o,
                    ):
                        slab_no = 0
                        for t in range(NQT):
                            nchunks = 4 * (t + 1)
                            slabs = []
                            for g0 in range(0, nchunks, SLAB):
                                gsz = min(SLAB, nchunks - g0)
                                e_slab = epool.tile(
                                    [128, SLAB, HPC, QT], F16, tag="e", name="e_slab"
                                )
                                n0 = TILE_IDX[(t, g0)]
                                src = e_in[n0 : n0 + gsz].rearrange(
                                    "n p h q -> p n h q"
                                )
                                if slab_no % 2 == 0:
                                    nc.sync.dma_start(e_slab[:, 0:gsz, :, :], src)
                                else:
                                    nc.gpsimd.dma_start(e_slab[:, 0:gsz, :, :], src)
                                slab_no += 1
                                slabs.append(e_slab)
                            ps_o = [
                                p2o.tile([HD + 1, QT], F32, tag=f"po{h}", name=f"po{h}")
                                for h in range(HPC)
                            ]
                            for c in range(nchunks):
                                e_slab = slabs[c // SLAB]
                                c_loc = c % SLAB
                                ps_s2 = p2s.tile([128, 2 * QT], F32, tag="ps2")
                                for h in range(HPC):
                                    nc.tensor.matmul(
                                        ps_s2[:, h * QT : (h + 1) * QT],
                                        kT_sb[
                                            h * HD : (h + 1) * HD,
                                            c // 4,
                                            (c % 4) * 128 : (c % 4 + 1) * 128,
                                        ],
                                        qT_sb[h * HD : (h + 1) * HD, t, :],
                                        start=True,
                                        stop=True,
                                    )
                                z_sb = zap.tile([128, HPC, QT], F16, tag="z")
                                nc.scalar.activation(
                                    z_sb[:].rearrange("p h q -> p (h q)"),
                                    ps_s2[:],
                                    mybir.ActivationFunctionType.Exp,
                                    bias=nbias[:],
                                )
                                a_sb = zap.tile([128, HPC, QT], F16, tag="a")
                                nc.vector.tensor_mul(
                                    a_sb[:], z_sb[:], e_slab[:, c_loc, :, :]
                                )
                                for h in range(HPC):
                                    nc.tensor.matmul(
                                        ps_o[h][:],
                                        v_sb[h][:, c, :],
                                        a_sb[:, h, :],
                                        start=(c == 0),
                                        stop=(c == nchunks - 1),
                                    )
                            ot_sb = otp.tile([HD + 1, HPC, QT], F16, tag="ot")
                            for h in range(HPC):
                                nc.vector.tensor_copy(ot_sb[:, h, :], ps_o[h][:])
                            nc.sync.dma_start(
                                a2a_in[t].rearrange("(h p) q -> p h q", h=HPC),
                                ot_sb[:],
                            )

                if phases >= 2.5:
                    nc.gpsimd.collective_compute(
                        "AllToAll",
                        mybir.AluOpType.bypass,
                        replica_groups=[list(range(NC))],
                        ins=[a2a_in.opt()],
                        outs=[a2a_out.opt()],
                    )

                # ------------- phase 3: normalize + o_proj ------------------
                if phases >= 3:
                    with (
                        tc.tile_pool(name="p3", bufs=1) as p3,
                        tc.tile_pool(name="p3y", bufs=2) as p3y,
                        tc.tile_pool(name="p3ps", bufs=2, space="PSUM") as p3ps,
                    ):
                        # den row order: l*8 + ci  (head h = 2*ci + l)
                        den_sb = p3.tile([NH, QT], F16, tag="den")
                        oT_sb = p3.tile([128, 8, QT], F16, tag="oT")
                        for l in range(HPC):
                            nc.sync.dma_start(
                                den_sb[l * 8 : (l + 1) * 8, :],
                                a2a_out[:, l * (HD + 1) + HD, :],
                            )
                            nc.sync.dma_start(
                                oT_sb[l * HD : (l + 1) * HD, :, :],
                                a2a_out[:, l * (HD + 1) : l * (HD + 1) + HD, :].rearrange(
                                    "c d q -> d c q"
                                ),
                            )
                        rden_sb = p3.tile([NH, QT], F32, tag="rden")
                        nc.vector.reciprocal(rden_sb[:], den_sb[:])

                        on_sb = p3.tile([128, 8, QT], F32R, tag="on")
                        for ci in range(8):
                            ps_b = p3ps.tile([128, QT], F32, tag="bc")
                            nc.tensor.matmul(
                                ps_b[:],
                                sel_sb[:, ci * 128 : (ci + 1) * 128],
                                rden_sb[:],
                                start=True,
                                stop=True,
                            )
                            nc.vector.tensor_mul(
                                on_sb[:, ci, :], oT_sb[:, ci, :], ps_b[:]
                            )

                        for qb in range(4):
                            y_sb = p3y.tile([128, H], F32, tag="y")
                            for nh2 in range(2):
                                ps_y = p3ps.tile([128, QT], F32, tag="py")
                                for c in range(8):
                                    nc.tensor.matmul(
                                        ps_y[:],
                                        on_sb[:, c, qb * 128 : (qb + 1) * 128],
                                        wo_sb[:, c, nh2 * QT : (nh2 + 1) * QT],
                                        start=(c == 0),
                                        stop=(c == 7),
                                    )
                                nc.vector.tensor_add(
                                    y_sb[:, nh2 * QT : (nh2 + 1) * QT],
                                    ps_y[:],
                                    bo_sb[:, nh2 * QT : (nh2 + 1) * QT],
                                )
                            nc.sync.dma_start(
                                y_out[qb * 128 : (qb + 1) * 128, :], y_sb[:]
                            )

            if phases < 3:
                with tc.tile_pool(name="dbg", bufs=1) as dbgp:
                    dbg = dbgp.tile([128, 2, QT], F32, tag="dbg")
                    nc.vector.tensor_copy(dbg[:], qT_sb[:, 0:2, :].bitcast(F32))
                    nc.sync.dma_start(
                        y_out[0:128, :], dbg[:].rearrange("p a b -> p (a b)")
                    )

    nc.compile()
    return nc


_PROGRAM_CACHE = {}


def _get_program():
    if "nc" not in _PROGRAM_CACHE:
        _PROGRAM_CACHE["nc"] = _build_program()
    return _PROGRAM_CACHE["nc"]


def _host_prep(x, idx, valid, geo_bias, Wq, Wk, Wv, Wo, bo):
    x2 = np.ascontiguousarray(np.asarray(x, dtype=np.float32).reshape(S, H))
    idx = np.asarray(idx).astype(np.int64)
    valid = np.asarray(valid).astype(bool)
    geo = np.asarray(geo_bias, dtype=np.float32)
    Wq = np.asarray(Wq, dtype=np.float32)
    Wk = np.asarray(Wk, dtype=np.float32)
    Wv = np.asarray(Wv, dtype=np.float32)
    Wo = np.asarray(Wo, dtype=np.float32)
    bo = np.asarray(bo, dtype=np.float32)

    qpos = np.arange(S, dtype=np.int64)[:, None]
    keep = valid & (idx <= qpos) & (idx >= 0)
    s_flat = idx[keep]
    q_flat = np.broadcast_to(qpos, idx.shape)[keep]
    lin = s_flat * S + q_flat

    bo_rep = np.ascontiguousarray(np.broadcast_to(bo[None, :], (128, H)))

    # den row order in phase 3 is r = l*8 + ci for head h = 2*ci + l
    sel16 = np.zeros((NH, H), dtype=np.float32)
    ch = np.arange(H)
    sel16[((ch // HD) % 2) * 8 + ch // 128, ch] = 1.0

    wq_scaled = Wq / np.sqrt(HD)
    x_bf = x2.astype(ml_dtypes.bfloat16)

    in_maps = []
    for core in range(NC):
        e_pack = np.empty((N_TILES, SC, HPC, QT), dtype=np.float16)
        for l in range(HPC):
            h = HPC * core + l
            w = np.exp(geo[h][keep].astype(np.float64))
            eT = np.bincount(lin, weights=w, minlength=S * S).reshape(S, S)
            for n, (t, c) in enumerate(TILE_LIST):
                e_pack[n, :, l, :] = eT[
                    c * SC : (c + 1) * SC, t * QT : (t + 1) * QT
                ].astype(np.float16)
        cs = slice(128 * core, 128 * (core + 1))
        in_maps.append(
            {
                "x": x_bf,
                "wq": np.ascontiguousarray(wq_scaled[:, cs]).astype(ml_dtypes.bfloat16),
                "wk": np.ascontiguousarray(Wk[:, cs]).astype(ml_dtypes.bfloat16),
                "wv": np.ascontiguousarray(Wv[:, cs]).astype(ml_dtypes.bfloat16),
                "wo": Wo,
                "bo_rep": bo_rep,
                "e_pack": e_pack,
                "sel16": sel16,
            }
        )
    return in_maps


LAST_RESULTS = None


def kernel(x, idx, valid, geo_bias, Wq, Wk, Wv, Wo, bo):
    global LAST_RESULTS
    b, s, h = np.asarray(x).shape
    assert (b, s, h) == (1, S, H)
    in_maps = _host_prep(x, idx, valid, geo_bias, Wq, Wk, Wv, Wo, bo)
    nc = _get_program()
    res = run_bass_kernel_spmd(nc, in_maps, core_ids=list(range(NC)))
    LAST_RESULTS = res
    y = np.concatenate([res.results[c]["y_part"] for c in range(NC)], axis=0)
    return y.reshape(1, S, H).astype(np.float32)



# revision 4
# speedup vs baseline: 1.2094x; 1.2094x over previous
"""Sparse attention (per-query top-K) Trainium2 kernel, 8-core tensor-parallel.

Strategy (heads sharded 2-per-core, dense-score formulation):
  - Host folds idx/valid/geo_bias into per-(s,q) merged bias factors
    E[s,q] = sum_{j: idx[q,j]==s} exp(geo_bias[h,q,j]), stored as causal
    fp16 tiles.  This turns the per-query gather/softmax into dense math:
        A^T = E^T * exp(S^T - C),   S^T = K @ Q^T (feature-major)
        out^T = [V | 1]^T @ A^T     (row 64 = softmax denominator)
  - Per core: Q/K/V projections for its 2 heads (x pre-transposed and
    cast to bf16 on host; projections in bf16 with fp32 accumulation),
    dense causal S^T on PE in bf16 — the two heads' 64-contraction
    matmuls run concurrently in row-halves of the PE array — exp on ACT
    (both heads per instruction via a 2-bank PSUM tile), E-multiply on
    DVE (fp16, two chunks per instruction), AV on PE (fp16).
  - AllToAll reshards the (unnormalized) head outputs + denominators so
    each core owns 512 query rows with all 16 heads, normalizes, and
    computes its o_proj row-slice with the full Wo in bf16.  Host
    concatenates.
"""

import sys

sys.path.insert(0, "/opt/trn_rl_repo")

import numpy as np
import ml_dtypes

from concourse import bacc, mybir, tile
from concourse.bass_utils import run_bass_kernel_spmd
from concourse.masks import make_identity

F32 = mybir.dt.float32
F32R = mybir.dt.float32r
F16 = mybir.dt.float16
BF16 = mybir.dt.bfloat16

S = 4096
H = 1024
NH = 16
KSEL = 32
HD = 64
NC = 8
HPC = NH // NC  # 2 heads per core
QT = 512
NQT = S // QT
SC = 128
CSHIFT = 2.0
SLAB = 16  # s-chunks per E-tile DMA slab

TILE_LIST = [(t, c) for t in range(NQT) for c in range(4 * (t + 1))]
N_TILES = len(TILE_LIST)  # 144
TILE_IDX = {tc: n for n, tc in enumerate(TILE_LIST)}


def _build_program(phases=3, n_reps=1, n_cores_build=NC):
    nc = bacc.Bacc(
        "TRN2", target_bir_lowering=False, debug=False, num_devices=n_cores_build
    )

    # host-pretransposed x: [st, p, hc, 512] (feature-major, bf16)
    xT_in = nc.dram_tensor("xT", [NQT, 128, 8, QT], BF16, kind="ExternalInput").ap()
    wq_in = nc.dram_tensor("wq", [128, 8, 128], BF16, kind="ExternalInput").ap()
    wk_in = nc.dram_tensor("wk", [128, 8, 128], BF16, kind="ExternalInput").ap()
    wv_in = nc.dram_tensor("wv", [128, 8, 128], BF16, kind="ExternalInput").ap()
    wo_in = nc.dram_tensor("wo", [128, 8, H], BF16, kind="ExternalInput").ap()
    bo_in = nc.dram_tensor("bo_rep", [128, H], F32, kind="ExternalInput").ap()
    e_in = nc.dram_tensor(
        "e_pack", [N_TILES, SC, HPC, QT], F16, kind="ExternalInput"
    ).ap()
    sel_in = nc.dram_tensor("sel16", [NH, H], F32, kind="ExternalInput").ap()
    y_out = nc.dram_tensor("y_part", [QT, H], F32, kind="ExternalOutput").ap()

    with tile.TileContext(nc) as tc:
        with (
            tc.tile_pool(name="const", bufs=1) as constp,
            tc.tile_pool(name="persist", bufs=1) as persist,
            tc.tile_pool(name="dram", bufs=1, space="DRAM") as dram,
        ):
            ident = constp.tile([128, 128], F32, tag="ident")
            make_identity(nc, ident[:])
            nbias = constp.tile([128, 1], F32, tag="nbias")
            nc.gpsimd.memset(nbias[:], -CSHIFT)

            wq_sb = constp.tile([128, 8, 128], BF16, tag="wq")
            wk_sb = constp.tile([128, 8, 128], BF16, tag="wk")
            wv_sb = constp.tile([128, 8, 128], BF16, tag="wv")
            nc.sync.dma_start(wq_sb[:], wq_in[:])
            nc.sync.dma_start(wk_sb[:], wk_in[:])
            nc.sync.dma_start(wv_sb[:], wv_in[:])
            wo_sb = constp.tile([128, 8, H], BF16, tag="wo")
            nc.sync.dma_start(wo_sb[:], wo_in[:])
            bo_sb = constp.tile([128, H], F32, tag="bo")
            nc.sync.dma_start(bo_sb[:], bo_in[:])
            sel_sb = constp.tile([NH, H], F32, tag="sel")
            nc.sync.dma_start(sel_sb[:], sel_in[:])

            qT_sb = persist.tile([128, NQT, QT], BF16, tag="qT")
            kT_sb = persist.tile([128, NQT, QT], BF16, tag="kT")
            v_sb = [
                persist.tile([128, S // SC, HD + 1], F16, tag=f"v{h}", name=f"v{h}")
                for h in range(HPC)
            ]
            for h in range(HPC):
                nc.gpsimd.memset(v_sb[h][:], 1.0)

            a2a_in = dram.tile([NC, HPC * (HD + 1), QT], F16)
            a2a_out = dram.tile([NC, HPC * (HD + 1), QT], F16)

            for _rep in range(n_reps):
                # ------------- phase 1: projections (feature-major) ---------
                with (
                    tc.tile_pool(name="xT", bufs=1) as xTp,
                    tc.tile_pool(name="vtmp", bufs=2) as vtmpp,
                    tc.tile_pool(name="p1ps", bufs=3, space="PSUM") as p1ps,
                    tc.tile_pool(name="p1projps", bufs=1, space="PSUM") as p1pp,
                ):
                    xT_full = xTp.tile([128, NQT, 8, QT], BF16, tag="xTf")
                    for st in range(NQT):
                        nc.sync.dma_start(xT_full[:, st, :, :], xT_in[st])
                    for st in range(NQT):
                        ps_q = p1pp.tile([128, QT], F32, tag="psq")
                        ps_k = p1pp.tile([128, QT], F32, tag="psk")
                        ps_v = p1pp.tile([128, QT], F32, tag="psv")
                        for c in range(8):
                            nc.tensor.matmul(
                                ps_q[:], wq_sb[:, c, :], xT_full[:, st, c, :],
                                start=(c == 0), stop=(c == 7),
                            )
                            nc.tensor.matmul(
                                ps_k[:], wk_sb[:, c, :], xT_full[:, st, c, :],
                                start=(c == 0), stop=(c == 7),
                            )
                            nc.tensor.matmul(
                                ps_v[:], wv_sb[:, c, :], xT_full[:, st, c, :],
                                start=(c == 0), stop=(c == 7),
                            )
                        nc.vector.tensor_copy(qT_sb[:, st, :], ps_q[:])
                        nc.vector.tensor_copy(kT_sb[:, st, :], ps_k[:])
                        vT_tmp = vtmpp.tile([128, QT], F32, tag="vt")
                        nc.scalar.copy(vT_tmp[:], ps_v[:])
                        ps_tv = p1ps.tile([128, QT], F32, tag="tp")
                        for i in range(4):
                            nc.tensor.transpose(
                                ps_tv[:, i * 128 : (i + 1) * 128],
                                vT_tmp[:, i * 128 : (i + 1) * 128],
                                ident[:],
                            )
                        ps_tv4 = ps_tv[:].rearrange("p (i h d) -> p i h d", i=4, h=HPC)
                        for h in range(HPC):
                            nc.vector.tensor_copy(
                                v_sb[h][:, st * 4 : (st + 1) * 4, 0:HD],
                                ps_tv4[:, :, h, :],
                            )

                # ------------- phase 2: attention ---------------------------
                if phases >= 2:
                    with (
                        tc.tile_pool(name="zap", bufs=3) as zap,
                        tc.tile_pool(name="ep", bufs=2) as epool,
                        tc.tile_pool(name="otp", bufs=2) as otp,
                        tc.tile_pool(name="p2s", bufs=3, space="PSUM") as p2s,
                        tc.tile_pool(name="p2o", bufs=1, space="PSUM") as p2o,
                    ):
                        slab_no = 0
                        for t in range(NQT):
                            nchunks = 4 * (t + 1)
                            slabs = []
                            for g0 in range(0, nchunks, SLAB):
                                gsz = min(SLAB, nchunks - g0)
                                e_slab = epool.tile(
                                    [128, SLAB, HPC, QT], F16, tag="e", name="e_slab"
                                )
                                n0 = TILE_IDX[(t, g0)]
                                src = e_in[n0 : n0 + gsz].rearrange(
                                    "n p h q -> p n h q"
                                )
                                if slab_no % 2 == 0:
                                    nc.sync.dma_start(e_slab[:, 0:gsz, :, :], src)
                                else:
                                    nc.gpsimd.dma_start(e_slab[:, 0:gsz, :, :], src)
                                slab_no += 1
                                slabs.append(e_slab)
                            ps_o = [
                                p2o.tile([HD + 1, QT], F32, tag=f"po{h}", name=f"po{h}")
                                for h in range(HPC)
                            ]
                            # chunks processed in pairs: the DVE E-multiply
                            # runs once per pair (amortizes instr overhead);
                            # pair never crosses a slab (SLAB, nchunks even)
                            for c0 in range(0, nchunks, 2):
                                e_slab = slabs[c0 // SLAB]
                                z_sb = zap.tile(
                                    [128, 2, HPC, QT], F16, tag="z", name="z2"
                                )
                                a_sb = zap.tile(
                                    [128, 2, HPC, QT], F16, tag="a", name="a2"
                                )
                                for pc in range(2):
                                    c = c0 + pc
                                    ps_s2 = p2s.tile([128, 2 * QT], F32, tag="ps2")
                                    for h in range(HPC):
                                        nc.tensor.matmul(
                                            ps_s2[:, h * QT : (h + 1) * QT],
                                            kT_sb[
                                                h * HD : (h + 1) * HD,
                                                c // 4,
                                                (c % 4) * 128 : (c % 4 + 1) * 128,
                                            ],
                                            qT_sb[h * HD : (h + 1) * HD, t, :],
                                            start=True,
                                            stop=True,
                                        )
                                    nc.scalar.activation(
                                        z_sb[:, pc, :, :].rearrange("p h q -> p (h q)"),
                                        ps_s2[:],
                                        mybir.ActivationFunctionType.Exp,
                                        bias=nbias[:],
                                    )
                                c_loc = c0 % SLAB
                                nc.vector.tensor_mul(
                                    a_sb[:],
                                    z_sb[:],
                                    e_slab[:, c_loc : c_loc + 2, :, :],
                                )
                                for pc in range(2):
                                    c = c0 + pc
                                    for h in range(HPC):
                                        nc.tensor.matmul(
                                            ps_o[h][:],
                                            v_sb[h][:, c, :],
                                            a_sb[:, pc, h, :],
                                            start=(c == 0),
                                            stop=(c == nchunks - 1),
                                        )
                            ot_sb = otp.tile([HD + 1, HPC, QT], F16, tag="ot")
                            for h in range(HPC):
                                nc.vector.tensor_copy(ot_sb[:, h, :], ps_o[h][:])
                            nc.sync.dma_start(
                                a2a_in[t].rearrange("(h p) q -> p h q", h=HPC),
                                ot_sb[:],
                            )

                if phases >= 2.5:
                    nc.gpsimd.collective_compute(
                        "AllToAll",
                        mybir.AluOpType.bypass,
                        replica_groups=[list(range(NC))],
                        ins=[a2a_in.opt()],
                        outs=[a2a_out.opt()],
                    )

                # ------------- phase 3: normalize + o_proj ------------------
                if phases >= 3:
                    with (
                        tc.tile_pool(name="p3", bufs=1) as p3,
                        tc.tile_pool(name="p3y", bufs=2) as p3y,
                        tc.tile_pool(name="p3ps", bufs=2, space="PSUM") as p3ps,
                    ):
                        # den row order: l*8 + ci  (head h = 2*ci + l)
                        den_sb = p3.tile([NH, QT], F16, tag="den")
                        oT_sb = p3.tile([128, 8, QT], F16, tag="oT")
                        for l in range(HPC):
                            nc.sync.dma_start(
                                den_sb[l * 8 : (l + 1) * 8, :],
                                a2a_out[:, l * (HD + 1) + HD, :],
                            )
                            nc.sync.dma_start(
                                oT_sb[l * HD : (l + 1) * HD, :, :],
                                a2a_out[:, l * (HD + 1) : l * (HD + 1) + HD, :].rearrange(
                                    "c d q -> d c q"
                                ),
                            )
                        rden_sb = p3.tile([NH, QT], F32, tag="rden")
                        nc.vector.reciprocal(rden_sb[:], den_sb[:])

                        on_sb = p3.tile([128, 8, QT], BF16, tag="on")
                        for ci in range(8):
                            ps_b = p3ps.tile([128, QT], F32, tag="bc")
                            nc.tensor.matmul(
                                ps_b[:],
                                sel_sb[:, ci * 128 : (ci + 1) * 128],
                                rden_sb[:],
                                start=True,
                                stop=True,
                            )
                            nc.vector.tensor_mul(
                                on_sb[:, ci, :], oT_sb[:, ci, :], ps_b[:]
                            )

                        for qb in range(4):
                            y_sb = p3y.tile([128, H], F32, tag="y")
                            for nh2 in range(2):
                                ps_y = p3ps.tile([128, QT], F32, tag="py")
                                for c in range(8):
                                    nc.tensor.matmul(
                                        ps_y[:],
                                        on_sb[:, c, qb * 128 : (qb + 1) * 128],
                                        wo_sb[:, c, nh2 * QT : (nh2 + 1) * QT],
                                        start=(c == 0),
                                        stop=(c == 7),
                                    )
                                nc.vector.tensor_add(
                                    y_sb[:, nh2 * QT : (nh2 + 1) * QT],
                                    ps_y[:],
                                    bo_sb[:, nh2 * QT : (nh2 + 1) * QT],
                                )
                            nc.sync.dma_start(
                                y_out[qb * 128 : (qb + 1) * 128, :], y_sb[:]
                            )

            if phases < 3:
                with tc.tile_pool(name="dbg", bufs=1) as dbgp:
                    dbg = dbgp.tile([128, 2, QT], F32, tag="dbg")
                    nc.vector.tensor_copy(dbg[:], qT_sb[:, 0:2, :].bitcast(F32))
                    nc.sync.dma_start(
                        y_out[0:128, :], dbg[:].rearrange("p a b -> p (a b)")
                    )

    nc.compile()
    return nc


_PROGRAM_CACHE = {}


def _get_program():
    if "nc" not in _PROGRAM_CACHE:
        _PROGRAM_CACHE["nc"] = _build_program()
    return _PROGRAM_CACHE["nc"]


def _host_prep(x, idx, valid, geo_bias, Wq, Wk, Wv, Wo, bo):
    x2 = np.ascontiguousarray(np.asarray(x, dtype=np.float32).reshape(S, H))
    idx = np.asarray(idx).astype(np.int64)
    valid = np.asarray(valid).astype(bool)
    geo = np.asarray(geo_bias, dtype=np.float32)
    Wq = np.asarray(Wq, dtype=np.float32)
    Wk = np.asarray(Wk, dtype=np.float32)
    Wv = np.asarray(Wv, dtype=np.float32)
    Wo = np.asarray(Wo, dtype=np.float32)
    bo = np.asarray(bo, dtype=np.float32)

    qpos = np.arange(S, dtype=np.int64)[:, None]
    keep = valid & (idx <= qpos) & (idx >= 0)
    s_flat = idx[keep]
    q_flat = np.broadcast_to(qpos, idx.shape)[keep]
    lin = s_flat * S + q_flat

    bo_rep = np.ascontiguousarray(np.broadcast_to(bo[None, :], (128, H)))

    # den row order in phase 3 is r = l*8 + ci for head h = 2*ci + l
    sel16 = np.zeros((NH, H), dtype=np.float32)
    ch = np.arange(H)
    sel16[((ch // HD) % 2) * 8 + ch // 128, ch] = 1.0

    wq_scaled = Wq / np.sqrt(HD)

    # x pre-transposed: [st, p, hc, 512] with value x[st*512+s, hc*128+p]
    xT = np.ascontiguousarray(
        x2.T.reshape(8, 128, NQT, QT).transpose(2, 1, 0, 3)
    ).astype(ml_dtypes.bfloat16)

    def wslice(W, core):
        cs = slice(128 * core, 128 * (core + 1))
        # [p, c, m] = W[c*128+p, cs][...]
        return np.ascontiguousarray(
            W[:, cs].reshape(8, 128, 128).transpose(1, 0, 2)
        ).astype(ml_dtypes.bfloat16)

    wo_t = np.ascontiguousarray(Wo.reshape(8, 128, H).transpose(1, 0, 2)).astype(
        ml_dtypes.bfloat16
    )

    in_maps = []
    for core in range(NC):
        e_pack = np.empty((N_TILES, SC, HPC, QT), dtype=np.float16)
        for l in range(HPC):
            h = HPC * core + l
            w = np.exp(geo[h][keep].astype(np.float64))
            eT = np.bincount(lin, weights=w, minlength=S * S).reshape(S, S)
            for n, (t, c) in enumerate(TILE_LIST):
                e_pack[n, :, l, :] = eT[
                    c * SC : (c + 1) * SC, t * QT : (t + 1) * QT
                ].astype(np.float16)
        in_maps.append(
            {
                "xT": xT,
                "wq": wslice(wq_scaled, core),
                "wk": wslice(Wk, core),
                "wv": wslice(Wv, core),
                "wo": wo_t,
                "bo_rep": bo_rep,
                "e_pack": e_pack,
                "sel16": sel16,
            }
        )
    return in_maps


LAST_RESULTS = None


def kernel(x, idx, valid, geo_bias, Wq, Wk, Wv, Wo, bo):
    global LAST_RESULTS
    b, s, h = np.asarray(x).shape
    assert (b, s, h) == (1, S, H)
    in_maps = _host_prep(x, idx, valid, geo_bias, Wq, Wk, Wv, Wo, bo)
    nc = _get_program()
    res = run_bass_kernel_spmd(nc, in_maps, core_ids=list(range(NC)))
    LAST_RESULTS = res
    y = np.concatenate([res.results[c]["y_part"] for c in range(NC)], axis=0)
    return y.reshape(1, S, H).astype(np.float32)
